# revision 16
# baseline (speedup 1.0000x reference)
"""BoT multi-head attention block (QKV proj + content/position attention +
out-proj + residual + LayerNorm) on 8 Trainium2 NeuronCores.

Sharding: tensor-parallel over heads (4 heads/core) x batch (2 batches, 4
cores each).  Each core computes q/k/v projections for its 256 feature
columns, full attention for its 4 heads, and a partial out-projection;
partials are summed with a ReduceScatter over each 4-core batch group, after
which each core applies residual + LayerNorm to its 512 rows.

Key layout trick: attention logits are computed TRANSPOSED (j on partitions,
i free) so the softmax numerator matmul (P^T as moving operand, V as
stationary) needs no transpose of the probability matrix; an extra all-ones
column in the stationary V supplies the softmax denominator for free.
"""

import contextlib
import os
import sys

os.environ.setdefault("MYCRO_LOCAL_CACHE", "1")
for _p in ("/opt/trn_rl_repo",):
    if os.path.isdir(_p) and _p not in sys.path:
        sys.path.append(_p)

import numpy as np

import concourse.bass as bass
from concourse import bacc
import concourse.mybir as mybir
import concourse.tile as tile
from concourse.bass_utils import run_bass_kernel_spmd

FP = mybir.dt.float32
FPR = mybir.dt.float32r
AF = mybir.ActivationFunctionType

B, N, D, H = 2, 2048, 1024, 16
NCORES = 8
GRP = 4                # cores per batch group
HPC = H // GRP         # heads per core = 4
C = D // GRP           # feature cols per core = 256
R = N // GRP           # output rows per core = 512
DH = D // H            # head dim = 64
SCALE = DH ** -0.5
LN_EPS = 1e-5

NT = N // 128          # 16 row tiles
KD = D // 128          # 8 contraction tiles over D
NS = N // 512          # 4 i-slices


def build():
    nc = bacc.Bacc("TRN2", target_bir_lowering=False, num_devices=NCORES)

    xT_t = nc.dram_tensor("xT", [D, N], FP, kind="ExternalInput")
    posT_t = nc.dram_tensor("posT", [C, N], FP, kind="ExternalInput")
    wq_t = nc.dram_tensor("wq", [D, C], FP, kind="ExternalInput")
    wk_t = nc.dram_tensor("wk", [D, C], FP, kind="ExternalInput")
    wv_t = nc.dram_tensor("wv", [D, C], FP, kind="ExternalInput")
    wo_t = nc.dram_tensor("wo", [C, D], FP, kind="ExternalInput")
    res_t = nc.dram_tensor("resid", [R, D], FP, kind="ExternalInput")
    g_t = nc.dram_tensor("ln_g", [D], FP, kind="ExternalInput")
    bt_t = nc.dram_tensor("ln_b", [D], FP, kind="ExternalInput")
    out_t = nc.dram_tensor("out", [R, D], FP, kind="ExternalOutput")

    res_tiles = res_t.ap().rearrange("(t p) d -> t p d", p=128)
    out_tiles = out_t.ap().rearrange("(t p) d -> t p d", p=128)

    def bcast_ap(ap, parts):
        return bass.AP(tensor=ap.tensor, offset=ap.offset,
                       ap=[[0, parts]] + list(ap.ap))

    with tile.TileContext(nc) as tc, contextlib.ExitStack() as ctx:
        persist = ctx.enter_context(tc.tile_pool(name="persist", bufs=1))
        attnp = ctx.enter_context(tc.tile_pool(name="attnp", bufs=1))
        psP = ctx.enter_context(tc.tile_pool(name="psP", bufs=4, space="PSUM"))
        psC = ctx.enter_context(tc.tile_pool(name="psC", bufs=2, space="PSUM"))
        dram = ctx.enter_context(tc.tile_pool(name="dram", bufs=1, space="DRAM"))

        ones64 = persist.tile([1, DH], FP, tag="ones64")
        nc.vector.memset(ones64, 1.0)
        onescol = persist.tile([128, 1], FP, tag="onescol")
        nc.vector.memset(onescol, 1.0)

        # ---------------- phase 1-2: load (pre-transposed on host), project
        with contextlib.ExitStack() as ph12_ctx:
            p12 = ph12_ctx.enter_context(tc.tile_pool(name="ph12", bufs=1))

            # weights + xT, cast to fp32r during DMA (SWDGE)
            wq_sb = p12.tile([128, KD, C], FPR, tag="wq")
            wk_sb = p12.tile([128, KD, C], FPR, tag="wk")
            wv_sb = p12.tile([128, KD, C], FPR, tag="wv")
            nc.gpsimd.dma_start(out=wq_sb, in_=wq_t.ap().rearrange("(k p) c -> p k c", p=128))
            nc.gpsimd.dma_start(out=wk_sb, in_=wk_t.ap().rearrange("(k p) c -> p k c", p=128))
            nc.gpsimd.dma_start(out=wv_sb, in_=wv_t.ap().rearrange("(k p) c -> p k c", p=128))

            xT_sb = p12.tile([128, KD, N], FPR, tag="xT")
            xT_src = xT_t.ap().rearrange("(k p) n -> p k n", p=128)
            for k in range(KD):
                nc.gpsimd.dma_start(out=xT_sb[:, k, :], in_=xT_src[:, k, :])
            xT = [xT_sb[:, k, :] for k in range(KD)]

            posT_sb = p12.tile([128, 2, N], FP, tag="posT")
            nc.sync.dma_start(out=posT_sb,
                              in_=posT_t.ap().rearrange("(m p) n -> p m n", p=128))
            posT = [posT_sb[:, m, :] for m in range(2)]

            # projections: qT/kpT [128 c, N] fp32r (head pair hp at rows 64*(h%2))
            qT = [attnp.tile([128, N], FPR, name=f"qT{m}", tag=f"qT{m}") for m in range(2)]
            kpT = [attnp.tile([128, N], FPR, name=f"kpT{m}", tag=f"kpT{m}") for m in range(2)]
            for m in range(2):
                for s in range(NS):
                    q_ps = psP.tile([128, 512], FP, tag="ps")
                    for k in range(KD):
                        nc.tensor.matmul(q_ps, wq_sb[:, k, m * 128:(m + 1) * 128],
                                         xT[k][:, s * 512:(s + 1) * 512],
                                         start=(k == 0), stop=(k == KD - 1))
                    nc.vector.tensor_copy(out=qT[m][:, s * 512:(s + 1) * 512], in_=q_ps)
                for s in range(NS):
                    kp_ps = psP.tile([128, 512], FP, tag="ps")
                    for k in range(KD):
                        nc.tensor.matmul(kp_ps, wk_sb[:, k, m * 128:(m + 1) * 128],
                                         xT[k][:, s * 512:(s + 1) * 512],
                                         start=(k == 0), stop=(k == KD - 1))
                    nc.vector.tensor_add(out=kpT[m][:, s * 512:(s + 1) * 512],
                                         in0=kp_ps, in1=posT[m][:, s * 512:(s + 1) * 512])

            # V natural [128 j, HPC, DH+1] fp32r, last col = ones (softmax denom)
            V = [attnp.tile([128, HPC, DH + 1], FPR, name=f"V{t}", tag=f"V{t}") for t in range(NT)]
            for t in range(NT):
                v_ps = psP.tile([128, C], FP, tag="ps")
                for k in range(KD):
                    nc.tensor.matmul(v_ps, xT[k][:, t * 128:(t + 1) * 128], wv_sb[:, k, :],
                                     start=(k == 0), stop=(k == KD - 1))
                nc.vector.tensor_copy(out=V[t][:, :, 0:DH],
                                      in_=v_ps.rearrange("p (h d) -> p h d", h=HPC))
                nc.vector.tensor_copy(
                    out=V[t][:, :, DH:DH + 1],
                    in_=onescol.broadcast_to([128, HPC, 1]))

        sbA = ctx.enter_context(tc.tile_pool(name="sbA", bufs=3))
        sbB = ctx.enter_context(tc.tile_pool(name="sbB", bufs=2))

        # ---------------- phase 3: attention ------------------------------
        # ST[j, i] = (k+p)q^T logits transposed; exp; OT[dv, i] += V^T-style
        OT = [attnp.tile([128, N], FPR, name=f"OT{m}", tag=f"OT{m}") for m in range(2)]
        for hp in range(2):
            for s in range(NS):
                ot_e = psP.tile([128, 512], FP, tag="ps")
                ot_o = psP.tile([128, 512], FP, tag="ps")
                for jt in range(NT):
                    st = psC.tile([128, 1024], FP, tag="st")
                    nc.tensor.matmul(st[:, 0:512],
                                     kpT[hp][0:64, jt * 128:(jt + 1) * 128],
                                     qT[hp][0:64, s * 512:(s + 1) * 512],
                                     start=True, stop=True)
                    nc.tensor.matmul(st[:, 512:1024],
                                     kpT[hp][64:128, jt * 128:(jt + 1) * 128],
                                     qT[hp][64:128, s * 512:(s + 1) * 512],
                                     start=True, stop=True)
                    ste = sbA.tile([128, 1024], FPR, tag="ste")
                    nc.scalar.activation(out=ste, in_=st, func=AF.Exp, scale=SCALE)
                    nc.tensor.matmul(ot_e[0:DH + 1, :], V[jt][:, 2 * hp, :],
                                     ste[:, 0:512],
                                     start=(jt == 0), stop=(jt == NT - 1))
                    nc.tensor.matmul(ot_o[0:DH + 1, :], V[jt][:, 2 * hp + 1, :],
                                     ste[:, 512:1024],
                                     start=(jt == 0), stop=(jt == NT - 1))
                # normalize: OT rows/colsum -> per-i reciprocal, bcast, multiply
                for par, ot in ((0, ot_e), (1, ot_o)):
                    cs = sbA.tile([1, 512], FP, tag="cs")
                    nc.vector.reciprocal(out=cs, in_=ot[DH:DH + 1, :])
                    rec_ps = psP.tile([128, 512], FP, tag="ps")
                    nc.tensor.matmul(rec_ps[0:DH, :], ones64, cs,
                                     start=True, stop=True)
                    rec = sbA.tile([DH, 512], FP, tag="rec")
                    nc.vector.tensor_copy(out=rec, in_=rec_ps[0:DH, :])
                    nc.vector.tensor_mul(
                        out=OT[hp][par * 64:par * 64 + DH, s * 512:(s + 1) * 512],
                        in0=ot[0:DH, :], in1=rec)

        # ---------------- phase 4: out-proj + ReduceScatter + LayerNorm ---
        wo_sb = persist.tile([128, 2, D], FPR, tag="wo")
        nc.gpsimd.dma_start(out=wo_sb, in_=wo_t.ap().rearrange("(k p) d -> p k d", p=128))

        oph = [dram.tile([N, 512], FP, name=f"oph{nh}", tag=f"oph{nh}") for nh in range(2)]
        rsh = [dram.tile([R, 512], FP, name=f"rsh{nh}", tag=f"rsh{nh}") for nh in range(2)]
        for nh in range(2):
            for it in range(NT):
                op_ps = psP.tile([128, 512], FP, tag="ps")
                for kt in range(2):
                    nc.tensor.matmul(op_ps, OT[kt][:, it * 128:(it + 1) * 128],
                                     wo_sb[:, kt, nh * 512:(nh + 1) * 512],
                                     start=(kt == 0), stop=(kt == 1))
                op_sb = sbB.tile([128, 512], FP, tag="op")
                nc.vector.tensor_copy(out=op_sb, in_=op_ps)
                nc.sync.dma_start(
                    out=oph[nh][:].rearrange("(t p) d -> t p d", p=128)[it],
                    in_=op_sb)
            nc.gpsimd.collective_compute(
                "ReduceScatter", mybir.AluOpType.add,
                replica_groups=[[0, 1, 2, 3], [4, 5, 6, 7]],
                ins=[oph[nh].opt()], outs=[rsh[nh].opt()])

        g_sb = persist.tile([128, D], FP, tag="g")
        b_sb = persist.tile([128, D], FP, tag="b")
        nc.gpsimd.dma_start(out=g_sb, in_=bcast_ap(g_t.ap(), 128))
        nc.gpsimd.dma_start(out=b_sb, in_=bcast_ap(bt_t.ap(), 128))
        eps_sb = persist.tile([128, 1], FP, tag="eps")
        nc.vector.memset(eps_sb, LN_EPS)

        for t in range(R // 128):
            xr = sbB.tile([128, D], FP, tag="xr")
            rd = sbB.tile([128, D], FP, tag="rd")
            nc.sync.dma_start(out=rd, in_=res_tiles[t])
            for nh in range(2):
                rs_sb = sbB.tile([128, 512], FP, tag="rsld")
                nc.sync.dma_start(
                    out=rs_sb,
                    in_=rsh[nh][:].rearrange("(t p) d -> t p d", p=128)[t])
                nc.vector.tensor_add(out=xr[:, nh * 512:(nh + 1) * 512],
                                     in0=rs_sb, in1=rd[:, nh * 512:(nh + 1) * 512])
            stats = sbB.tile([128, 2, 6], FP, tag="stats")
            mv = sbB.tile([128, 2], FP, tag="mv")
            nc.vector.bn_stats(out=stats[:, 0, :], in_=xr[:, 0:512])
            nc.vector.bn_stats(out=stats[:, 1, :], in_=xr[:, 512:1024])
            nc.vector.bn_aggr(out=mv, in_=stats)
            nc.scalar.activation(out=mv[:, 1:2], in_=mv[:, 1:2], func=AF.Sqrt,
                                 bias=eps_sb, scale=1.0)
            nc.vector.reciprocal(out=mv[:, 1:2], in_=mv[:, 1:2])
            nc.vector.tensor_scalar(out=xr, in0=xr,
                                    scalar1=mv[:, 0:1], scalar2=mv[:, 1:2],
                                    op0=mybir.AluOpType.subtract,
                                    op1=mybir.AluOpType.mult)
            nc.vector.tensor_mul(out=xr, in0=xr, in1=g_sb)
            nc.vector.tensor_add(out=xr, in0=xr, in1=b_sb)
            nc.sync.dma_start(out=out_tiles[t], in_=xr)

    nc.compile()
    return nc


_NC = None
_last_in_maps = None


def kernel(**inputs) -> np.ndarray:
    global _NC, _last_in_maps
    if _NC is None:
        _NC = build()
    nc = _NC

    q_s = np.asarray(inputs["q_s"], np.float32)
    pos = np.asarray(inputs["pos_emb"], np.float32)
    Wq = np.asarray(inputs["Wq"], np.float32)
    Wk = np.asarray(inputs["Wk"], np.float32)
    Wv = np.asarray(inputs["Wv"], np.float32)
    Wo = np.asarray(inputs["Wo"], np.float32)
    bo = np.asarray(inputs["bo"], np.float32)
    ln_g = np.asarray(inputs["ln_g"], np.float32)
    ln_b = np.asarray(inputs["ln_b"], np.float32)

    in_maps = []
    for c in range(NCORES):
        b, g = divmod(c, GRP)
        cs = slice(g * C, (g + 1) * C)
        rows = slice(g * R, (g + 1) * R)
        in_maps.append({
            "xT": np.ascontiguousarray(q_s[b].T),
            "posT": np.ascontiguousarray(pos[b][:, cs].T),
            "wq": np.ascontiguousarray(Wq[:, cs]),
            "wk": np.ascontiguousarray(Wk[:, cs]),
            "wv": np.ascontiguousarray(Wv[:, cs]),
            "wo": np.ascontiguousarray(Wo[cs, :]),
            "resid": np.ascontiguousarray(q_s[b][rows, :] + bo[None, :]),
            "ln_g": ln_g,
            "ln_b": ln_b,
        })

    _last_in_maps = in_maps
    res = run_bass_kernel_spmd(nc, in_maps, list(range(NCORES)))
    out = np.empty((B, N, D), np.float32)
    for c in range(NCORES):
        b, g = divmod(c, GRP)
        out[b, g * R:(g + 1) * R, :] = res.results[c]["out"]
    return out


# revision 20
# speedup vs baseline: 1.1965x; 1.1965x over previous
"""BoT multi-head attention block (QKV proj + content/position attention +
out-proj + residual + LayerNorm) on 8 Trainium2 NeuronCores.

Sharding: tensor-parallel over heads (4 heads/core) x batch (2 batches, 4
cores each).  Each core computes q/k/v projections for its 256 feature
columns, full attention for its 4 heads, and a partial out-projection;
partials are summed with row-chunked ReduceScatters over each 4-core batch
group (overlapped with attention of later chunks), after which each core
applies residual + LayerNorm to its 4x128 rows.

Layout trick: attention logits are computed TRANSPOSED (j on partitions, i
free) so the softmax numerator matmul (P^T moving, V stationary) needs no
transpose of the probability matrix; an extra all-ones column in the
stationary V supplies the softmax denominator for free.  Host passes x and
pos pre-transposed.  Projections/out-proj run in fp32r, attention matmuls
in bf16 (1 cycle/col vs 2 for fp32r).
"""

import contextlib
import os
import sys

os.environ.setdefault("MYCRO_LOCAL_CACHE", "1")
for _p in ("/opt/trn_rl_repo",):
    if os.path.isdir(_p) and _p not in sys.path:
        sys.path.append(_p)

import numpy as np

import concourse.bass as bass
from concourse import bacc
import concourse.mybir as mybir
import concourse.tile as tile
from concourse.bass_utils import run_bass_kernel_spmd

FP = mybir.dt.float32
FPR = mybir.dt.float32r
BF = mybir.dt.bfloat16
AF = mybir.ActivationFunctionType

B, N, D, H = 2, 2048, 1024, 16
NCORES = 8
GRP = 4                # cores per batch group
HPC = H // GRP         # heads per core = 4
C = D // GRP           # feature cols per core = 256
R = N // GRP           # output rows per core = 512
DH = D // H            # head dim = 64
SCALE = DH ** -0.5
LN_EPS = 1e-5

NT = N // 128          # 16 row tiles
KD = D // 128          # 8 contraction tiles over D
NS = N // 512          # 4 i-slices

ATT_DT = BF            # dtype of attention matmul operands


def build():
    nc = bacc.Bacc("TRN2", target_bir_lowering=False, num_devices=NCORES)

    # fp32r-typed inputs are plain fp32 bits; typing them fp32r lets HWDGE
    # load them with no cast while satisfying the fp32r-producer rule.
    xT_t = nc.dram_tensor("xT", [D, N], FPR, kind="ExternalInput")
    posT_t = nc.dram_tensor("posT", [C, N], FP, kind="ExternalInput")
    wq_t = nc.dram_tensor("wq", [D, C], FPR, kind="ExternalInput")
    wk_t = nc.dram_tensor("wk", [D, C], FPR, kind="ExternalInput")
    wv_t = nc.dram_tensor("wv", [D, C], FPR, kind="ExternalInput")
    wo_t = nc.dram_tensor("wo", [C, D], FPR, kind="ExternalInput")
    res_t = nc.dram_tensor("resid", [R, D], FP, kind="ExternalInput")
    g_t = nc.dram_tensor("ln_g", [D], FP, kind="ExternalInput")
    bt_t = nc.dram_tensor("ln_b", [D], FP, kind="ExternalInput")
    out_t = nc.dram_tensor("out", [R, D], FP, kind="ExternalOutput")

    res_tiles = res_t.ap().rearrange("(t p) d -> t p d", p=128)
    out_tiles = out_t.ap().rearrange("(t p) d -> t p d", p=128)

    def bcast_ap(ap, parts):
        return bass.AP(tensor=ap.tensor, offset=ap.offset,
                       ap=[[0, parts]] + list(ap.ap))

    with tile.TileContext(nc) as tc, contextlib.ExitStack() as ctx:
        persist = ctx.enter_context(tc.tile_pool(name="persist", bufs=1))
        attnp = ctx.enter_context(tc.tile_pool(name="attnp", bufs=1))
        psP = ctx.enter_context(tc.tile_pool(name="psP", bufs=4, space="PSUM"))
        psC = ctx.enter_context(tc.tile_pool(name="psC", bufs=2, space="PSUM"))
        dram = ctx.enter_context(tc.tile_pool(name="dram", bufs=1, space="DRAM"))

        ones64 = persist.tile([1, DH], FP, tag="ones64")
        nc.vector.memset(ones64, 1.0)
        onescol = persist.tile([128, 1], FP, tag="onescol")
        nc.vector.memset(onescol, 1.0)

        sbA = ctx.enter_context(tc.tile_pool(name="sbA", bufs=3))

        # ---------------- phase 1-2: load (pre-transposed on host), project
        ph12_ctx = contextlib.ExitStack()
        p12 = ph12_ctx.enter_context(tc.tile_pool(name="ph12", bufs=1))

        wq_sb = p12.tile([128, KD, C], FPR, tag="wq")
        wk_sb = p12.tile([128, KD, C], FPR, tag="wk")
        wv_sb = p12.tile([128, KD, C], FPR, tag="wv")
        xT_sb = p12.tile([128, KD, N], FPR, tag="xT")
        xT_src = xT_t.ap().rearrange("(k p) n -> p k n", p=128)
        nc.sync.dma_start(out=wq_sb, in_=wq_t.ap().rearrange("(k p) c -> p k c", p=128))
        for k in range(KD):
            nc.sync.dma_start(out=xT_sb[:, k, :], in_=xT_src[:, k, :])
        nc.sync.dma_start(out=wk_sb, in_=wk_t.ap().rearrange("(k p) c -> p k c", p=128))
        nc.sync.dma_start(out=wv_sb, in_=wv_t.ap().rearrange("(k p) c -> p k c", p=128))
        xT = [xT_sb[:, k, :] for k in range(KD)]

        posT_sb = p12.tile([128, 2, N], FP, tag="posT")
        nc.sync.dma_start(out=posT_sb,
                          in_=posT_t.ap().rearrange("(m p) n -> p m n", p=128))
        posT = [posT_sb[:, m, :] for m in range(2)]

        # projections: qT/kpT [128 c, N] (head pair hp at rows 64*(h%2))
        qT = [attnp.tile([128, N], ATT_DT, name=f"qT{m}", tag=f"qT{m}") for m in range(2)]
        kpT = [attnp.tile([128, N], ATT_DT, name=f"kpT{m}", tag=f"kpT{m}") for m in range(2)]
        V = [attnp.tile([128, HPC, DH + 1], ATT_DT, name=f"V{t}", tag=f"V{t}")
             for t in range(NT)]

        def proj_qkp(m):
            for s in range(NS):
                q_ps = psP.tile([128, 512], FP, tag="ps", name="q_ps")
                for k in range(KD):
                    nc.tensor.matmul(q_ps, wq_sb[:, k, m * 128:(m + 1) * 128],
                                     xT[k][:, s * 512:(s + 1) * 512],
                                     start=(k == 0), stop=(k == KD - 1))
                nc.vector.tensor_copy(out=qT[m][:, s * 512:(s + 1) * 512], in_=q_ps)
            for s in range(NS):
                kp_ps = psP.tile([128, 512], FP, tag="ps", name="kp_ps")
                for k in range(KD):
                    nc.tensor.matmul(kp_ps, wk_sb[:, k, m * 128:(m + 1) * 128],
                                     xT[k][:, s * 512:(s + 1) * 512],
                                     start=(k == 0), stop=(k == KD - 1))
                nc.vector.tensor_add(out=kpT[m][:, s * 512:(s + 1) * 512],
                                     in0=kp_ps, in1=posT[m][:, s * 512:(s + 1) * 512])

        proj_qkp(0)
        for t in range(NT):
            v_ps = psP.tile([128, C], FP, tag="ps", name="v_ps")
            for k in range(KD):
                nc.tensor.matmul(v_ps, xT[k][:, t * 128:(t + 1) * 128], wv_sb[:, k, :],
                                 start=(k == 0), stop=(k == KD - 1))
            nc.vector.tensor_copy(out=V[t][:, :, 0:DH],
                                  in_=v_ps.rearrange("p (h d) -> p h d", h=HPC))
            nc.vector.tensor_copy(out=V[t][:, :, DH:DH + 1],
                                  in_=onescol.broadcast_to([128, HPC, 1]))

        # ---------------- phases 3-4 interleaved per i-slice s -------------
        pools = {}

        wo_sb = persist.tile([128, 2, D], FPR, tag="wo")
        nc.sync.dma_start(out=wo_sb, in_=wo_t.ap().rearrange("(k p) d -> p k d", p=128))
        g_sb = persist.tile([128, D], FP, tag="g")
        b_sb = persist.tile([128, D], FP, tag="b")
        nc.gpsimd.dma_start(out=g_sb, in_=bcast_ap(g_t.ap(), 128))
        nc.gpsimd.dma_start(out=b_sb, in_=bcast_ap(bt_t.ap(), 128))
        eps_sb = persist.tile([128, 1], FP, tag="eps")
        nc.vector.memset(eps_sb, LN_EPS)

        OT = [attnp.tile([128, N], FPR, name=f"OT{m}", tag=f"OT{m}") for m in range(2)]
        oph = [dram.tile([R, D], FP, name=f"oph{s}", tag=f"oph{s}") for s in range(NS)]
        rsh = [dram.tile([128, D], FP, name=f"rsh{s}", tag=f"rsh{s}") for s in range(NS)]

        def attention(s, hp):
            ot_e = psP.tile([128, 512], FP, tag="ps", name="ot_e")
            ot_o = psP.tile([128, 512], FP, tag="ps", name="ot_o")
            for jt in range(NT):
                st = psC.tile([128, 1024], FP, tag="st", name="st")
                nc.tensor.matmul(st[:, 0:512],
                                 kpT[hp][0:64, jt * 128:(jt + 1) * 128],
                                 qT[hp][0:64, s * 512:(s + 1) * 512],
                                 start=True, stop=True)
                nc.tensor.matmul(st[:, 512:1024],
                                 kpT[hp][64:128, jt * 128:(jt + 1) * 128],
                                 qT[hp][64:128, s * 512:(s + 1) * 512],
                                 start=True, stop=True)
                ste = sbA.tile([128, 1024], ATT_DT, tag="ste", name="ste")
                nc.scalar.activation(out=ste, in_=st, func=AF.Exp, scale=SCALE)
                nc.tensor.matmul(ot_e[0:DH + 1, :], V[jt][:, 2 * hp, :],
                                 ste[:, 0:512],
                                 start=(jt == 0), stop=(jt == NT - 1))
                nc.tensor.matmul(ot_o[0:DH + 1, :], V[jt][:, 2 * hp + 1, :],
                                 ste[:, 512:1024],
                                 start=(jt == 0), stop=(jt == NT - 1))
            # normalize: per-i reciprocal of colsum row, bcast via PE, multiply
            for par, ot in ((0, ot_e), (1, ot_o)):
                cs = sbA.tile([1, 512], FP, tag="cs", name="cs")
                nc.vector.reciprocal(out=cs, in_=ot[DH:DH + 1, :])
                rec_ps = psP.tile([128, 512], FP, tag="ps", name="rec_ps")
                nc.tensor.matmul(rec_ps[0:DH, :], ones64, cs, start=True, stop=True)
                rec = sbA.tile([DH, 512], FP, tag="rec", name="rec")
                nc.vector.tensor_copy(out=rec, in_=rec_ps[0:DH, :])
                nc.vector.tensor_mul(
                    out=OT[hp][par * 64:par * 64 + DH, s * 512:(s + 1) * 512],
                    in0=ot[0:DH, :], in1=rec)

        def outproj_rs_ln(s):
            sbB = pools["sbB"]
            # partial out-projection for this slice's 4 row blocks
            for it4 in range(4):
                it = s * 4 + it4
                op_sb = sbB.tile([128, D], FP, tag="op", name="op_sb")
                for nh in range(2):
                    op_ps = psP.tile([128, 512], FP, tag="ps", name="op_ps")
                    for kt in range(2):
                        nc.tensor.matmul(op_ps, OT[kt][:, it * 128:(it + 1) * 128],
                                         wo_sb[:, kt, nh * 512:(nh + 1) * 512],
                                         start=(kt == 0), stop=(kt == 1))
                    nc.vector.tensor_copy(out=op_sb[:, nh * 512:(nh + 1) * 512],
                                          in_=op_ps)
                nc.sync.dma_start(
                    out=oph[s][:].rearrange("(t p) d -> t p d", p=128)[it4],
                    in_=op_sb)
            nc.gpsimd.collective_compute(
                "ReduceScatter", mybir.AluOpType.add,
                replica_groups=[[0, 1, 2, 3], [4, 5, 6, 7]],
                ins=[oph[s].opt()], outs=[rsh[s].opt()])
            # residual + LayerNorm on this core's 128-row chunk
            xr = sbB.tile([128, D], FP, tag="xr", name="xr")
            rd = sbB.tile([128, D], FP, tag="rd", name="rd")
            rs_sb = sbB.tile([128, D], FP, tag="rsld", name="rs_sb")
            nc.sync.dma_start(out=rd, in_=res_tiles[s])
            nc.sync.dma_start(out=rs_sb, in_=rsh[s][:])
            nc.vector.tensor_add(out=xr, in0=rs_sb, in1=rd)
            stats = sbB.tile([128, 2, 6], FP, tag="stats", name="stats")
            mv = sbB.tile([128, 2], FP, tag="mv", name="mv")
            nc.vector.bn_stats(out=stats[:, 0, :], in_=xr[:, 0:512])
            nc.vector.bn_stats(out=stats[:, 1, :], in_=xr[:, 512:1024])
            nc.vector.bn_aggr(out=mv, in_=stats)
            nc.scalar.activation(out=mv[:, 1:2], in_=mv[:, 1:2], func=AF.Sqrt,
                                 bias=eps_sb, scale=1.0)
            nc.vector.reciprocal(out=mv[:, 1:2], in_=mv[:, 1:2])
            nc.vector.tensor_scalar(out=xr, in0=xr,
                                    scalar1=mv[:, 0:1], scalar2=mv[:, 1:2],
                                    op0=mybir.AluOpType.subtract,
                                    op1=mybir.AluOpType.mult)
            nc.vector.tensor_mul(out=xr, in0=xr, in1=g_sb)
            nc.vector.tensor_add(out=xr, in0=xr, in1=b_sb)
            nc.sync.dma_start(out=out_tiles[s], in_=xr)

        for s in range(NS):
            attention(s, 0)
            if s == 0:
                proj_qkp(1)  # overlaps first attention slice on other engines
            attention(s, 1)
            if s == 0:
                # x/pos/weight staging no longer needed; free its SBUF before
                # opening the out-proj/LN pool
                ph12_ctx.close()
                pools["sbB"] = ctx.enter_context(tc.tile_pool(name="sbB", bufs=2))
            outproj_rs_ln(s)

    nc.compile()
    return nc


_NC = None
_last_in_maps = None


def kernel(**inputs) -> np.ndarray:
    global _NC, _last_in_maps
    if _NC is None:
        _NC = build()
    nc = _NC

    q_s = np.asarray(inputs["q_s"], np.float32)
    pos = np.asarray(inputs["pos_emb"], np.float32)
    Wq = np.asarray(inputs["Wq"], np.float32)
    Wk = np.asarray(inputs["Wk"], np.float32)
    Wv = np.asarray(inputs["Wv"], np.float32)
    Wo = np.asarray(inputs["Wo"], np.float32)
    bo = np.asarray(inputs["bo"], np.float32)
    ln_g = np.asarray(inputs["ln_g"], np.float32)
    ln_b = np.asarray(inputs["ln_b"], np.float32)

    in_maps = []
    for c in range(NCORES):
        b, g = divmod(c, GRP)
        cs = slice(g * C, (g + 1) * C)
        resid = np.concatenate(
            [q_s[b][512 * s + 128 * g: 512 * s + 128 * (g + 1)] for s in range(NS)],
            axis=0) + bo[None, :]
        in_maps.append({
            "xT": np.ascontiguousarray(q_s[b].T),
            "posT": np.ascontiguousarray(pos[b][:, cs].T),
            "wq": np.ascontiguousarray(Wq[:, cs]),
            "wk": np.ascontiguousarray(Wk[:, cs]),
            "wv": np.ascontiguousarray(Wv[:, cs]),
            "wo": np.ascontiguousarray(Wo[cs, :]),
            "resid": np.ascontiguousarray(resid),
            "ln_g": ln_g,
            "ln_b": ln_b,
        })

    _last_in_maps = in_maps
    res = run_bass_kernel_spmd(nc, in_maps, list(range(NCORES)))
    out = np.empty((B, N, D), np.float32)
    for c in range(NCORES):
        b, g = divmod(c, GRP)
        o = res.results[c]["out"]
        for s in range(NS):
            out[b, 512 * s + 128 * g: 512 * s + 128 * (g + 1), :] = \
                o[128 * s:128 * (s + 1)]
    return out
